# revision 23
# baseline (speedup 1.0000x reference)
"""HCHA 3-layer hypergraph conv on 8 TRN2 NeuronCores.

v3: aggregate-first for layers 1-2 (gather raw x / h tables, apply W after
pass2), W-first for layer 3. Gathers are issued round-robin on the 4 SWDGE
queues so descriptor generation runs on all four Q7 core pairs. One-hot
segment matrices are built in bf16 on DVE; psum copies / ELU pieces run on
the Scalar (ACT) engine.

Per layer (l=1,2):
  pass1: dma_gather tab[vi] (hi-sharded nnz, sorted by hi) -> one-hot PE
         segment-sum -> xe_raw_c = sum/deg_e -> AllGather(xe_raw)
  pass2: dma_gather xe_raw[hi] (vi-sharded nnz, sorted by vi) -> seg-sum
         -> agg = sum/deg_v -> transpose on PE -> h = ELU(agg @ W + b)
         l=1: h row-major -> AllGather(h) = next table
         l=2: h^T written directly into the stationary for layer 3
Layer 3 (W-first): msg = h2T.T @ W3 -> AllGather(msg) -> pass1 -> AG(xe)
  -> pass2 (+bias via deg*b matmul) -> out.
bf16 gather tables / matmul operands, fp32 psum accumulation.
int16 gather indices -> tables addressed via lo/hi base split at 32768.
"""
import sys
import types


def _setup_env():
    if "/opt/trn_rl_repo" not in sys.path:
        sys.path.insert(0, "/opt/trn_rl_repo")
    try:
        import antenv
        if "antenv.axon_hooks" not in sys.modules:
            mod = types.ModuleType("antenv.axon_hooks")
            _h = [None]
            mod.set_axon_ntff_profile_hook = lambda h: _h.__setitem__(0, h)
            mod.get_axon_ntff_profile_hook = lambda: _h[0]
            sys.modules["antenv.axon_hooks"] = mod
            antenv.axon_hooks = mod
            sys.path.insert(0, "/root/.axon_site/trn_agent_boot")
            import trn_boot
            mod.set_axon_ntff_profile_hook(
                trn_boot._ntff_profile_via_ctypes("/opt/axon/libaxon_pjrt.so")
            )
    except Exception:
        pass


_setup_env()

import numpy as np
import ml_dtypes

import concourse.bass as bass
import concourse.tile as tile
from concourse import bacc, mybir
from concourse.bass_utils import run_bass_kernel_spmd

N = 50000
NNZ = 800000
NC = 8
SH = N // NC          # 6250 rows per core
P = 128
G = (SH + P - 1) // P  # 49 psum groups per core
LAST = SH - (G - 1) * P  # 106 rows in last group
SPLIT = 32768
CB = 8               # gather-chunk size in 128-row blocks (ring limit!)
PAD_SEG = 999.0
# chunked-AllGather plan: 7 small chunks so the serialized collective chain
# starts early and keeps pace with pass compute (each ~0.45MB/rank -> mesh)
CH_END = [7, 14, 21, 28, 35, 42, 49]
RS = [0, 896, 1792, 2688, 3584, 4480, 5376]
RE = [896, 1792, 2688, 3584, 4480, 5376, 6250]
OB = [8 * r for r in RS]             # output base row in the gathered table

F32 = mybir.dt.float32
BF16 = mybir.dt.bfloat16
I16 = mybir.dt.int16
ADD = mybir.AluOpType.add
Copy = mybir.ActivationFunctionType.Copy
Relu = mybir.ActivationFunctionType.Relu
Exp = mybir.ActivationFunctionType.Exp


def _wrap_idx(idx):
    """stream idx (len%16==0) -> [128, len/16] int16 wrapped+replicated."""
    w = idx.reshape(-1, 16).T.astype(np.int16)  # [16, cols]
    return np.tile(w, (8, 1))


def _prep_pass(tgt, src):
    """Preprocess one scatter pass. tgt: per-nnz segment id (sharded dim),
    src: per-nnz gather row id. Returns per-core streams with identical
    shapes across cores (padded to per-group max block counts)."""
    order = np.argsort(tgt, kind="stable")
    tgt_s = tgt[order]
    src_s = src[order]
    core_of = tgt_s // SH
    rel = tgt_s - core_of * SH
    grp_of = rel // P
    seg_of = rel - grp_of * P
    bucket = (src_s >= SPLIT).astype(np.int64)

    lists = {}
    for c in range(NC):
        m = core_of == c
        gm, sm, bm, vm = grp_of[m], seg_of[m], bucket[m], src_s[m]
        for g in range(G):
            mg = gm == g
            for b in (0, 1):
                mb = mg & (bm == b)
                v_, s_ = vm[mb], sm[mb]
                o_ = np.argsort(v_, kind="stable")
                lists[(c, g, b)] = (v_[o_], s_[o_])

    nb = np.zeros((G, 2), np.int64)
    for g in range(G):
        for b in (0, 1):
            mx = max(len(lists[(c, g, b)][0]) for c in range(NC))
            nb[g, b] = max((mx + P - 1) // P, 1)

    T_lo = int(nb[:, 0].sum())
    T_hi = int(nb[:, 1].sum())
    T_lo_pad = ((T_lo + CB - 1) // CB) * CB
    T_hi_pad = ((T_hi + CB - 1) // CB) * CB

    out = {"nb_lo": nb[:, 0].tolist(), "nb_hi": nb[:, 1].tolist(),
           "Tlo": T_lo_pad, "Thi": T_hi_pad, "cores": []}
    bf = ml_dtypes.bfloat16
    for c in range(NC):
        res = {}
        for b, T, Tp in ((0, T_lo, T_lo_pad), (1, T_hi, T_hi_pad)):
            idx = np.zeros(Tp * P, np.int64)
            seg = np.full((Tp, P), PAD_SEG, np.float32)
            pos = 0
            for g in range(G):
                v, s = lists[(c, g, b)]
                nblk = nb[g, b]
                idx[pos * P : pos * P + len(v)] = v - (SPLIT if b else 0)
                seg.reshape(-1)[pos * P : pos * P + len(s)] = s
                pos += nblk
            res["idx_lo" if b == 0 else "idx_hi"] = _wrap_idx(idx)
            res["seg_lo" if b == 0 else "seg_hi"] = seg.T.astype(bf).copy()
        out["cores"].append(res)
    return out


def _build_program(p1, p2, nbmax):
    nc = bacc.Bacc("TRN2", target_bir_lowering=False, debug=False,
                   num_devices=NC, num_swdge_queues=4)
    d = {}
    def dram_in(name, shape, dt):
        d[name] = nc.dram_tensor(name, shape, dt, kind="ExternalInput").ap()
        return d[name]

    xtab = dram_in("xtab", [N, 256], BF16)
    W = [dram_in(f"W{l}", [256, 256 if l < 3 else 128], BF16) for l in (1, 2, 3)]
    brow = [dram_in(f"b{l}", [1, 256 if l < 3 else 128], BF16) for l in (1, 2, 3)]
    ones_in = dram_in("ones_row", [1, P], BF16)
    degv = dram_in("degv", [1, G * P], BF16)
    inv_e = dram_in("inv_e", [128, G], F32)
    inv_v = dram_in("inv_v", [128, G], F32)
    i1lo = dram_in("i1lo", [128, p1["Tlo"] * 8], I16)
    i1hi = dram_in("i1hi", [128, p1["Thi"] * 8], I16)
    i2lo = dram_in("i2lo", [128, p2["Tlo"] * 8], I16)
    i2hi = dram_in("i2hi", [128, p2["Thi"] * 8], I16)
    s1lo = dram_in("s1lo", [128, p1["Tlo"]], BF16)
    s1hi = dram_in("s1hi", [128, p1["Thi"]], BF16)
    s2lo = dram_in("s2lo", [128, p2["Tlo"]], BF16)
    s2hi = dram_in("s2hi", [128, p2["Thi"]], BF16)
    out_ap = nc.dram_tensor("out", [SH, 128], F32, kind="ExternalOutput").ap()

    # staging + AllGather buffers
    xb = [nc.dram_tensor(f"xb{l}", [SH, 256 if l < 3 else 128], BF16)
          for l in (1, 2, 3)]
    xe_full = [nc.dram_tensor(f"xef{l}", [N, 256 if l < 3 else 128], BF16,
                              addr_space="Shared") for l in (1, 2, 3)]
    hb1 = nc.dram_tensor("hb1", [SH, 256], BF16)
    hf1 = nc.dram_tensor("hf1", [N, 256], BF16, addr_space="Shared")
    mb3 = nc.dram_tensor("mb3", [SH, 128], BF16)
    msgf3 = nc.dram_tensor("msgf3", [N, 128], BF16, addr_space="Shared")
    wrm_in = nc.dram_tensor("wrm_in", [8, 64], BF16)
    wrm_out = nc.dram_tensor("wrm_out", [64, 64], BF16, addr_space="Shared")
    rg = [list(range(NC))]

    with tile.TileContext(nc) as tc:
        import contextlib
        ctx = contextlib.ExitStack()
        sb_const = ctx.enter_context(tc.tile_pool(name="const", bufs=1))
        sb_h = ctx.enter_context(tc.tile_pool(name="hT", bufs=1))
        sb_w = ctx.enter_context(tc.tile_pool(name="w", bufs=1))
        sb_g = ctx.enter_context(tc.tile_pool(name="gath", bufs=14))
        sb_s = ctx.enter_context(tc.tile_pool(name="smat", bufs=5))
        sb_io = ctx.enter_context(tc.tile_pool(name="io", bufs=4))
        sb_idx = ctx.enter_context(tc.tile_pool(name="idx", bufs=20))
        ps = ctx.enter_context(tc.tile_pool(name="ps", bufs=3, space="PSUM"))
        ps_mm = ctx.enter_context(tc.tile_pool(name="psmm", bufs=3, space="PSUM"))
        ps_tr = ctx.enter_context(tc.tile_pool(name="pstr", bufs=2, space="PSUM"))

        # warm up the collective path before the first real AllGather
        nc.gpsimd.collective_compute(
            "AllGather", mybir.AluOpType.bypass, replica_groups=rg,
            ins=[wrm_in.ap().opt()], outs=[wrm_out.ap().opt()])

        # constants
        iota3 = sb_const.tile([128, nbmax, 128], BF16)
        ii = sb_const.tile([128, nbmax * 128], mybir.dt.int32)
        nc.gpsimd.iota(ii[:], pattern=[[0, nbmax], [1, 128]], base=0,
                       channel_multiplier=0)
        nc.vector.tensor_copy(iota3[:].rearrange("p a b -> p (a b)"), ii[:])
        from concourse.masks import make_identity
        ident = sb_const.tile([128, 128], BF16)
        make_identity(nc, ident[:])
        degv_t = sb_const.tile([1, G * P], BF16)
        nc.sync.dma_start(degv_t[:], degv[:, :])
        ones_t = sb_const.tile([1, P], BF16)
        nc.sync.dma_start(ones_t[:], ones_in[:, :])
        inv_e_t = sb_const.tile([128, G], F32)
        nc.sync.dma_start(inv_e_t[:], inv_e[:, :])
        inv_v_t = sb_const.tile([128, G], F32)
        nc.sync.dma_start(inv_v_t[:], inv_v[:, :])
        segs = {}
        for nm, apx in (("s1lo", s1lo), ("s1hi", s1hi), ("s2lo", s2lo),
                        ("s2hi", s2hi)):
            t = sb_const.tile([128, apx.shape[1], 1], BF16, tag=nm, name=f"seg_{nm}")
            nc.sync.dma_start(t[:].rearrange("p a b -> p (a b)"), apx[:, :])
            segs[nm] = t

        # weights
        wt = {}
        bt = {}
        for l in (1, 2, 3):
            fout = 256 if l < 3 else 128
            wt[l] = [sb_w.tile([128, fout], BF16, tag=f"w{l}_{k}",
                               name=f"wt{l}_{k}") for k in range(2)]
            for k in range(2):
                nc.sync.dma_start(wt[l][k][:], W[l - 1][k * 128 : (k + 1) * 128, :])
            bt[l] = sb_w.tile([1, fout], BF16, tag=f"b{l}", name=f"bt{l}")
            nc.sync.dma_start(bt[l][:], brow[l - 1][:, :])

        # layer-3 stationary (h2T), written by layer-2 pass2
        hT = [sb_h.tile([128, SH], BF16, tag=f"hT_{k}", name=f"hT{k}")
              for k in range(2)]

        qctr = [0]  # global round-robin over the 4 SWDGE queues

        class Stream:
            """Pull-based gather stream: table rows -> SBUF blocks.
            idx tiles prefetch 4 chunks ahead of the gather consuming them."""
            def __init__(self, idx_ap, table_ap, fout, total_blocks, name):
                self.idx_ap, self.table_ap, self.fout = idx_ap, table_ap, fout
                self.total = total_blocks
                self.nchunks = (total_blocks + CB - 1) // CB
                self.name = name
                self.next_chunk = 0
                self.next_idx = 0
                self.tiles = {}
                self.idx_tiles = {}

            def _fetch_idx(self, upto):
                while self.next_idx <= min(upto, self.nchunks - 1):
                    cc = self.next_idx
                    it = sb_idx.tile([128, CB * 8], I16, tag="ix",
                                     name=f"ix_{self.name}_{cc}")
                    nc.sync.dma_start(it[:], self.idx_ap[:, cc * CB * 8 : (cc + 1) * CB * 8])
                    self.idx_tiles[cc] = it
                    self.next_idx += 1

            def block(self, j):
                ci = j // CB
                self._fetch_idx(ci + 4)
                while self.next_chunk <= ci:
                    cc = self.next_chunk
                    self._fetch_idx(cc)
                    it = self.idx_tiles.pop(cc)
                    gt = sb_g.tile([128, CB, self.fout], BF16, tag="g", name=f"g_{self.name}_{cc}")
                    nc.gpsimd.dma_gather(
                        out_ap=gt[:], in_ap=self.table_ap, idxs_ap=it[:],
                        num_idxs=CB * P, num_idxs_reg=CB * P, elem_size=self.fout,
                        queue_num=qctr[0] % 4)
                    qctr[0] += 1
                    self.tiles[cc] = gt
                    self.next_chunk += 1
                return self.tiles[ci][:, j % CB, :]

        def spass(table_ap, fout, pp, seg_lo, seg_hi, ilo, ihi, group_cb,
                  has_tail, pname, ag_hook=None):
            """One scatter pass: segment-sum gathered rows into G groups."""
            st_lo = Stream(ilo, table_ap[:, :], fout, pp["Tlo"], f"{pname}lo")
            st_hi = Stream(ihi, table_ap[SPLIT:, :], fout, pp["Thi"], f"{pname}hi")
            pos = [0, 0]
            pending = []

            def flush_one():
                gg, rr, pp_ = pending.pop(0)
                group_cb(gg, rr, pp_)
                if ag_hook is not None:
                    ag_hook(gg)

            for g in range(G):
                rows = LAST if g == G - 1 else P
                total = pp["nb_lo"][g] + pp["nb_hi"][g]
                done = 0
                psum = ps.tile([128, fout], F32, tag="ps")
                for b, st, sgt in ((0, st_lo, seg_lo), (1, st_hi, seg_hi)):
                    nblk = (pp["nb_lo"] if b == 0 else pp["nb_hi"])[g]
                    j0 = pos[b]
                    # batched one-hot build for the whole run (bf16)
                    S = sb_s.tile([128, nbmax, 128], BF16, tag="S")
                    nc.vector.tensor_tensor(
                        out=S[:, 0:nblk, :],
                        in0=sgt[:, j0 : j0 + nblk, :].broadcast_to([128, nblk, 128]),
                        in1=iota3[:, 0:nblk, :],
                        op=mybir.AluOpType.is_equal)
                    for k in range(nblk):
                        gblk = st.block(j0 + k)
                        last = (done == total - 1) and not has_tail
                        nc.tensor.matmul(psum[:], lhsT=S[:, k, :], rhs=gblk,
                                         start=(done == 0), stop=last)
                        done += 1
                    pos[b] += nblk
                # defer the heavy per-group callback two groups so its
                # cross-engine waits never reach a queue head before ready
                pending.append((g, rows, psum))
                if len(pending) > 2:
                    flush_one()
            while pending:
                flush_one()

        def transpose_to(xv, rows, tag):
            """xv [rows, 256] bf16 -> aggT [128, 2, rows] bf16 (k-chunks)."""
            aggT = sb_io.tile([128, 2, 128], BF16, tag=tag)
            for h in range(2):
                ptr = ps_tr.tile([128, 128], BF16, tag="tr")
                nc.tensor.transpose(out=ptr[:, 0:rows],
                                    in_=xv[0:rows, h * 128 : (h + 1) * 128],
                                    identity=ident[0:rows, 0:rows])
                nc.scalar.copy(aggT[:, h, 0:rows], ptr[:, 0:rows])
            return aggT

        def elu_into(dst_ap, psrc, rows, cols):
            """dst = ELU(psrc[0:rows, 0:cols]) via 3 ACT ops + 1 DVE op."""
            r = sb_io.tile([128, cols], F32, tag=f"er{cols}")
            nc.scalar.activation(r[0:rows, 0:cols], psrc, Relu, scale=-1.0)
            e = sb_io.tile([128, cols], F32, tag=f"ee{cols}")
            nc.scalar.activation(e[0:rows, 0:cols], r[0:rows, 0:cols], Exp,
                                 scale=-1.0)
            pos_t = sb_io.tile([128, cols], F32, tag=f"ep{cols}")
            nc.scalar.activation(pos_t[0:rows, 0:cols], psrc, Relu)
            nc.vector.scalar_tensor_tensor(
                out=dst_ap, in0=e[0:rows, 0:cols], scalar=-1.0,
                in1=pos_t[0:rows, 0:cols], op0=ADD, op1=ADD)

        def mk_ag(src_t, dst_t):
            def hook(g):
                for k in range(len(CH_END)):
                    if g == CH_END[k] - 1:
                        nc.gpsimd.collective_compute(
                            "AllGather", mybir.AluOpType.bypass,
                            replica_groups=rg,
                            ins=[src_t.ap()[RS[k] : RE[k], :].opt()],
                            outs=[dst_t.ap()[OB[k] : OB[k] + 8 * (RE[k] - RS[k]), :].opt()])
            return hook

        # ---------- shared pass-1 callback (xe_raw = psum/deg_e) ----------
        def mk_p1_cb(l, fout):
            def p1_cb(g, rows, psum):
                xt = sb_io.tile([128, fout], BF16, tag="xe")
                nc.scalar.activation(xt[0:rows, :], psum[0:rows, :], Copy,
                                     scale=inv_e_t[0:rows, g : g + 1])
                nc.sync.dma_start(xb[l - 1].ap()[g * P : g * P + rows, :],
                                  xt[0:rows, :])
            return p1_cb

        # ---------- layer 1: aggregate-first, h1 row-major ----------
        spass(xtab, 256, p1, segs["s1lo"], segs["s1hi"], i1lo, i1hi,
              mk_p1_cb(1, 256), has_tail=False, pname="a",
              ag_hook=mk_ag(xb[0], xe_full[0]))

        def p2_cb_l1(g, rows, psum):
            xv = sb_io.tile([128, 256], BF16, tag="xv")
            nc.scalar.activation(xv[0:rows, :], psum[0:rows, :], Copy,
                                 scale=inv_v_t[0:rows, g : g + 1])
            aggT = transpose_to(xv, rows, "aggT1")
            pm = ps_mm.tile([128, 256], F32, tag="pm")
            for k in range(2):
                nc.tensor.matmul(pm[0:rows, :], lhsT=aggT[:, k, 0:rows],
                                 rhs=wt[1][k][:], start=(k == 0), stop=False)
            nc.tensor.matmul(pm[0:rows, :], lhsT=ones_t[:, 0:rows],
                             rhs=bt[1][:], start=False, stop=True)
            ht = sb_io.tile([128, 256], BF16, tag="ht")
            elu_into(ht[0:rows, :], pm[0:rows, :], rows, 256)
            nc.sync.dma_start(hb1.ap()[g * P : g * P + rows, :], ht[0:rows, :])

        spass(xe_full[0].ap(), 256, p2, segs["s2lo"], segs["s2hi"], i2lo, i2hi,
              p2_cb_l1, has_tail=False, pname="b",
              ag_hook=mk_ag(hb1, hf1))

        # ---------- layer 2: aggregate-first, h2T into stationary ----------
        spass(hf1.ap(), 256, p1, segs["s1lo"], segs["s1hi"], i1lo, i1hi,
              mk_p1_cb(2, 256), has_tail=False, pname="c",
              ag_hook=mk_ag(xb[1], xe_full[1]))

        ag_mb3 = mk_ag(mb3, msgf3)

        def p2_cb_l2(g, rows, psum):
            xv = sb_io.tile([128, 256], BF16, tag="xv")
            nc.scalar.activation(xv[0:rows, :], psum[0:rows, :], Copy,
                                 scale=inv_v_t[0:rows, g : g + 1])
            aggT = transpose_to(xv, rows, "aggT2")
            for m in range(2):
                pt = ps_mm.tile([128, 256], F32, tag="pm", name=f"pt{m}")
                for k in range(2):
                    nc.tensor.matmul(
                        pt[:, 0:rows],
                        lhsT=wt[2][k][:, m * 128 : (m + 1) * 128],
                        rhs=aggT[:, k, 0:rows], start=(k == 0), stop=False)
                nc.tensor.matmul(pt[:, 0:rows],
                                 lhsT=bt[2][:, m * 128 : (m + 1) * 128],
                                 rhs=ones_t[:, 0:rows], start=False, stop=True)
                elu_into(hT[m][:, g * P : g * P + rows], pt[:, 0:rows], 128,
                         rows)
            # fused layer-3 A-step for this group (hT cols just written)
            pma = ps_mm.tile([128, 256], F32, tag="pm", name="pma")
            for k in range(2):
                nc.tensor.matmul(
                    pma[0:rows, 0:128],
                    lhsT=hT[k][:, g * P : g * P + rows],
                    rhs=wt[3][k][:], start=(k == 0), stop=(k == 1))
            mt = sb_io.tile([128, 128], BF16, tag="mt")
            nc.scalar.copy(mt[0:rows, :], pma[0:rows, 0:128])
            nc.sync.dma_start(mb3.ap()[g * P : g * P + rows, :],
                              mt[0:rows, :])
            ag_mb3(g)

        spass(xe_full[1].ap(), 256, p2, segs["s2lo"], segs["s2hi"], i2lo, i2hi,
              p2_cb_l2, has_tail=False, pname="d")

        # ---------- layer 3: W-first (A-step fused into p2_cb_l2) ----------
        spass(msgf3.ap(), 128, p1, segs["s1lo"], segs["s1hi"], i1lo, i1hi,
              mk_p1_cb(3, 128), has_tail=False, pname="e",
              ag_hook=mk_ag(xb[2], xe_full[2]))

        def p2_cb_l3(g, rows, psum):
            # bias: psum += degv[seg] * b  (k=1 matmul), then /deg_v
            nc.tensor.matmul(psum[0:rows, :],
                             lhsT=degv_t[:, g * P : g * P + rows],
                             rhs=bt[3][:], start=False, stop=True)
            ot = sb_io.tile([128, 128], F32, tag="o3")
            nc.scalar.activation(ot[0:rows, :], psum[0:rows, :], Copy,
                                 scale=inv_v_t[0:rows, g : g + 1])
            nc.sync.dma_start(out_ap[g * P : g * P + rows, :], ot[0:rows, :])

        spass(xe_full[2].ap(), 128, p2, segs["s2lo"], segs["s2hi"], i2lo, i2hi,
              p2_cb_l3, has_tail=True, pname="f")
        ctx.close()

    nc.compile()
    return nc


_CACHE = {}


def _balance(lo_all, hi_all):
    """Quantile (snake) striping of ids into psum groups: equalizes
    per-(group,bucket) edge counts within AND across cores."""
    relabel = np.empty(N, np.int64)
    for c in range(NC):
        ids = np.arange(c * SH, (c + 1) * SH)
        lo = lo_all[ids]
        hh = hi_all[ids]
        order = np.argsort(lo * 64 + hh, kind="stable")
        slots = np.full(G, P, np.int64)
        slots[G - 1] = LAST
        seq = []
        fwd = True
        while len(seq) < SH:
            rng = range(G) if fwd else range(G - 1, -1, -1)
            for k in rng:
                if slots[k] > 0:
                    seq.append(k)
                    slots[k] -= 1
                    if len(seq) == SH:
                        break
            fwd = not fwd
        assign = np.empty(SH, np.int64)
        assign[order] = np.asarray(seq)
        posg = np.zeros(G, np.int64)
        pos = np.empty(SH, np.int64)
        for i in range(SH):
            k = assign[i]
            pos[i] = k * P + posg[k]
            posg[k] += 1
        relabel[ids] = c * SH + pos
    return relabel


def _posmap():
    r = np.arange(N) % SH
    c = np.arange(N) // SH
    k = np.searchsorted(np.asarray(RE), r, side="right")
    rs = np.asarray(RS)[k]
    re = np.asarray(RE)[k]
    return np.asarray(OB)[k] + c * (re - rs) + (r - rs)


def kernel(x, edge_index, edge_weight, W1, b1, W2, b2, W3, b3):
    x = np.asarray(x, np.float32)
    vi = np.asarray(edge_index[0], np.int64)
    hi = np.asarray(edge_index[1], np.int64)
    deg_v = np.maximum(np.bincount(vi, minlength=N), 1).astype(np.float32)
    deg_e = np.maximum(np.bincount(hi, minlength=N), 1).astype(np.float32)

    pm_ = _posmap()
    # balanced relabel of nodes and hyperedges within their core shards
    rel_e = _balance(np.bincount(hi[pm_[vi] < SPLIT], minlength=N),
                     np.bincount(hi[pm_[vi] >= SPLIT], minlength=N))
    rel_v = _balance(np.bincount(vi[pm_[rel_e[hi]] < SPLIT], minlength=N),
                     np.bincount(vi[pm_[rel_e[hi]] >= SPLIT], minlength=N))
    rel_e = _balance(np.bincount(hi[pm_[rel_v[vi]] < SPLIT], minlength=N),
                     np.bincount(hi[pm_[rel_v[vi]] >= SPLIT], minlength=N))
    vin = rel_v[vi]
    hin = rel_e[hi]
    deg_v = deg_v[np.argsort(rel_v)]
    deg_e = deg_e[np.argsort(rel_e)]
    p1 = _prep_pass(hin, pm_[vin])  # segments=hedges, gather=nodes (remapped)
    p2 = _prep_pass(vin, pm_[hin])  # segments=nodes, gather=hedges (remapped)
    nbmax = max(max(p1["nb_lo"]), max(p1["nb_hi"]), max(p2["nb_lo"]),
                max(p2["nb_hi"]))

    key = (p1["Tlo"], p1["Thi"], p2["Tlo"], p2["Thi"],
           tuple(p1["nb_lo"]), tuple(p1["nb_hi"]),
           tuple(p2["nb_lo"]), tuple(p2["nb_hi"]))
    if key not in _CACHE:
        _CACHE[key] = _build_program(p1, p2, nbmax)
    nc = _CACHE[key]

    bf = ml_dtypes.bfloat16
    xtab = np.empty((N, 256), bf)
    xtab[pm_[rel_v]] = x.astype(bf)
    in_maps = []
    for c in range(NC):
        sl = slice(c * SH, (c + 1) * SH)
        m = {
            "xtab": xtab,
            "W1": W1.astype(bf), "W2": W2.astype(bf), "W3": W3.astype(bf),
            "b1": b1.reshape(1, -1).astype(bf),
            "b2": b2.reshape(1, -1).astype(bf),
            "b3": b3.reshape(1, -1).astype(bf),
            "ones_row": np.ones((1, P), bf),
            "degv": np.pad(deg_v[sl], (0, G * P - SH)).reshape(1, -1).astype(bf),
            "inv_e": np.pad(1.0 / deg_e[sl], (0, G * P - SH)).reshape(G, P).T.copy(),
            "inv_v": np.pad(1.0 / deg_v[sl], (0, G * P - SH)).reshape(G, P).T.copy(),
            "i1lo": p1["cores"][c]["idx_lo"], "i1hi": p1["cores"][c]["idx_hi"],
            "i2lo": p2["cores"][c]["idx_lo"], "i2hi": p2["cores"][c]["idx_hi"],
            "s1lo": p1["cores"][c]["seg_lo"], "s1hi": p1["cores"][c]["seg_hi"],
            "s2lo": p2["cores"][c]["seg_lo"], "s2hi": p2["cores"][c]["seg_hi"],
        }
        in_maps.append(m)

    trace = bool(int(__import__("os").environ.get("KERNEL_TRACE", "0")))
    res = run_bass_kernel_spmd(nc, in_maps, core_ids=list(range(NC)),
                               trace=trace)
    kernel.last_exec_time_ns = res.exec_time_ns
    out = np.concatenate([res.results[c]["out"] for c in range(NC)], axis=0)
    return out[rel_v].astype(np.float32)
